# revision 30
# baseline (speedup 1.0000x reference)
"""Trainium2 Bass kernel for DisentangledSpatialSA.

Reference computation (per batch b, with C=256, IC=128, N=64*64=4096):
    qkv = w_qkv @ x + b_qkv                    # [384, N]
    q, k, v = qkv split into 3 x [IC, N]
    k -= mean_n(k); q -= mean_n(q)             # per-channel spatial centering
    pw[i, j] = sum_c k[c, i] * q[c, j]
    pw = softmax(pw / (sqrt(IC) * TEMP), axis=j)
    y[c, i] = sum_j pw[i, j] * v[c, j]
    out = x + w_out @ y + b_out

Simplifications used (exact up to softmax shift invariance):
  - q centering and the q/k biases cancel inside the row softmax, so only k
    is centered and only v's bias is applied.
  - softmax max-subtraction is skipped: logits are ~N(0, 0.5), safely inside
    fp32 exp range.
  - normalization is applied after the PV matmul: y = (V e) / s, with the
    row sums s computed by a bf16 pairwise tree on VectorE plus one
    gpsimd.partition_all_reduce (which also broadcasts across partitions).

Sharding: data-parallel over batch, one batch element per NeuronCore (8).

Layout: everything channel-major with spatial flattened (n = 4096).
S_t[j, i] tiles are built with keys j on partitions (lhsT = q-tile, rhs = k~),
so the softmax denominators are partition-axis sums; PV uses lhsT = v^T tiles
(DMA transpose of bf16 v) and rhs = exp(S_t).
"""

import numpy as np

import concourse.bacc as bacc
import concourse.bass as bass
import concourse.tile as tile
from concourse import mybir
from concourse import bass_isa
from concourse.bass_utils import run_bass_kernel_spmd
from concourse.masks import make_identity

F32 = mybir.dt.float32
F32R = mybir.dt.float32r
BF16 = mybir.dt.bfloat16

CH = 256
IC = 128
N = 4096
TEMP = 0.05
SCALE = 1.0 / (np.sqrt(np.float32(IC)) * TEMP)  # applied inside exp

P = 128          # partitions
IMW = 1024       # i-macro tile width (query free dim per attention pass)
NMACRO = N // IMW
NJ = N // P      # 32 key tiles
MMF = 512        # max moving free dim per matmul


def build_bass() -> bass.Bass:
    nc = bacc.Bacc("TRN2", target_bir_lowering=False, debug=False, num_devices=8)

    # fp32r-typed external inputs: bits are fp32; fp32r lets matmuls consume
    # them at full (1 cycle/row) rate without an on-chip rounding pass.
    x_d = nc.dram_tensor("x", [CH, N], F32R, kind="ExternalInput")
    wqkvT_d = nc.dram_tensor("wqkvT", [CH, 3 * IC], F32R, kind="ExternalInput")
    bv_d = nc.dram_tensor("bv", [IC, 1], F32, kind="ExternalInput")
    woutT_d = nc.dram_tensor("woutT", [IC, CH], F32R, kind="ExternalInput")
    bout_d = nc.dram_tensor("bout", [CH, 1], F32, kind="ExternalInput")
    out_d = nc.dram_tensor("out", [CH, N], F32, kind="ExternalOutput")

    with tile.TileContext(nc) as tc:
        with (
            tc.tile_pool(name="big", bufs=1) as big,          # long-lived SBUF
            tc.tile_pool(name="small", bufs=1) as small,      # weights/bias
            tc.tile_pool(name="ework", bufs=8) as ework,      # exp tiles
            tc.tile_pool(name="tree", bufs=3) as treep,       # softmax-sum tree
            tc.tile_pool(name="norm", bufs=2) as normp,       # sums/recip
            tc.tile_pool(name="outp", bufs=4) as outp,        # output staging
            tc.tile_pool(name="spsum", bufs=2, space="PSUM") as spsum,  # 4 banks
            tc.tile_pool(name="ypsum", bufs=2, space="PSUM") as ypsum,  # 4 banks
        ):
            # ---------- load inputs ----------
            # Small tensors issue from the Scalar HWDGE queue, bulk x from the
            # Sync queue: dma_start issue costs ~1.3us each, so spreading
            # queues + issuing weights first gets the first matmul going early.
            W = []
            for cchunk in range(2):
                wt = small.tile([P, 3 * IC], F32R, tag=f"w{cchunk}")
                nc.scalar.dma_start(out=wt, in_=wqkvT_d[cchunk * P:(cchunk + 1) * P, :])
                W.append(wt)
            woutT = small.tile([IC, CH], F32R, tag="woutT")
            nc.scalar.dma_start(out=woutT, in_=woutT_d[:, :])
            bv = small.tile([IC, 1], F32, tag="bv")
            nc.scalar.dma_start(out=bv, in_=bv_d[:, :])
            bout = []
            for oc in range(2):
                bt = small.tile([P, 1], F32, tag=f"bout{oc}")
                nc.scalar.dma_start(out=bt, in_=bout_d[oc * P:(oc + 1) * P, :])
                bout.append(bt)
            ident_bf = small.tile([P, P], BF16, tag="ident")
            make_identity(nc, ident_bf)
            # ~3.4us of dependency-free matmuls: lifts the PE HAM clock gate
            # to 2.4 GHz before the real work lands
            warm_ps = spsum.tile([P, P], F32, tag="s")
            for _ in range(32):
                nc.tensor.matmul(warm_ps, ident_bf, ident_bf, start=True, stop=True)
            X = []
            for cchunk in range(2):
                xt = big.tile([P, N], F32R, tag=f"x{cchunk}")
                for h in range(2):
                    sl = slice(h * (N // 2), (h + 1) * (N // 2))
                    nc.sync.dma_start(
                        out=xt[:, sl], in_=x_d[cchunk * P:(cchunk + 1) * P, sl]
                    )
                X.append(xt)

            # ---------- QKV projection (k first: it gates attention start) --
            q_sb = big.tile([P, N], BF16, tag="q")
            k_sb = big.tile([P, N], F32, tag="k")
            v_bf = big.tile([P, N], BF16, tag="v")
            ksum8 = small.tile([P, N // MMF], F32, tag="ksum8")
            for m in (1, 0, 2):  # k, q, v
                for nt in range(N // MMF):
                    pool = ypsum if nt % 2 == 0 else spsum
                    ps = pool.tile([P, MMF], F32, tag="ypsum" if nt % 2 == 0 else "s")
                    sl = slice(nt * MMF, (nt + 1) * MMF)
                    for cchunk in range(2):
                        nc.tensor.matmul(
                            ps,
                            W[cchunk][:, m * IC:(m + 1) * IC],
                            X[cchunk][:, sl],
                            start=(cchunk == 0),
                            stop=(cchunk == 1),
                        )
                    if m == 0:
                        with nc.allow_low_precision("q used in bf16 logits"):
                            nc.vector.tensor_copy(q_sb[:, sl], ps)
                    elif m == 1:
                        nc.vector.tensor_copy(k_sb[:, sl], ps)
                        # fold the spatial sum into the pipeline per chunk
                        nc.vector.tensor_reduce(
                            out=ksum8[:, nt:nt + 1], in_=ps,
                            axis=mybir.AxisListType.X, op=mybir.AluOpType.add,
                        )
                    else:
                        nc.scalar.activation(
                            out=v_bf[:, sl], in_=ps,
                            func=mybir.ActivationFunctionType.Identity,
                            bias=bv, scale=1.0,
                        )

            # ---------- center k over spatial axis (write bf16) ----------
            kneg = small.tile([P, 1], F32, tag="kneg")
            nc.vector.tensor_reduce(
                out=kneg, in_=ksum8, axis=mybir.AxisListType.X, op=mybir.AluOpType.add
            )
            nc.vector.tensor_scalar_mul(kneg, kneg, -1.0 / N)
            kc_sb = big.tile([P, N], BF16, tag="kc")
            for h in range(4):
                sl = slice(h * (N // 4), (h + 1) * (N // 4))
                nc.scalar.activation(
                    out=kc_sb[:, sl], in_=k_sb[:, sl],
                    func=mybir.ActivationFunctionType.Identity,
                    bias=kneg, scale=1.0,
                )

            # ---------- v^T tiles via PE transpose (bf16) ----------
            vt = big.tile([P, NJ, IC], BF16, tag="vt")
            for jt in range(NJ):
                tps = spsum.tile([P, P], BF16, tag="s")
                nc.tensor.transpose(
                    tps, v_bf[:, jt * P:(jt + 1) * P], ident_bf
                )
                nc.vector.tensor_copy(vt[:, jt, :], tps)

            # ---------- output projection (emitted per-imacro, interleaved
            # into the NEXT imacro's loop so it never head-of-line blocks PE)
            y_tiles = []

            def emit_proj(im):
                isl = slice(im * IMW, (im + 1) * IMW)
                for oc in range(2):
                    pps = ypsum.tile([P, IMW], F32, tag="ypsum")
                    for h in range(IMW // MMF):
                        nc.tensor.matmul(
                            pps[:, h * MMF:(h + 1) * MMF],
                            woutT[:, oc * P:(oc + 1) * P],
                            y_tiles[im][:, h * MMF:(h + 1) * MMF],
                            start=True,
                            stop=True,
                        )
                    osb = outp.tile([P, IMW], F32, tag="osb")
                    nc.scalar.activation(
                        out=osb, in_=pps,
                        func=mybir.ActivationFunctionType.Identity,
                        bias=bout[oc], scale=1.0,
                    )
                    nc.vector.tensor_add(osb, osb, X[oc][:, isl].bitcast(F32))
                    nc.sync.dma_start(out=out_d[oc * P:(oc + 1) * P, isl], in_=osb)

            # ---------- attention (normalized y saved; projection deferred) --
            for im in range(NMACRO):
                yps = ypsum.tile([P, IMW], F32, tag="ypsum")
                levels: list = [None] * 8
                for jt in range(NJ):
                    if jt == 20 and im >= 1:
                        emit_proj(im - 1)
                    sps = spsum.tile([P, IMW], F32, tag="s")
                    for h in range(IMW // MMF):
                        nc.tensor.matmul(
                            sps[:, h * MMF:(h + 1) * MMF],
                            q_sb[:, jt * P:(jt + 1) * P],
                            kc_sb[:, im * IMW + h * MMF: im * IMW + (h + 1) * MMF],
                            start=True,
                            stop=True,
                        )
                    e = ework.tile([P, IMW], BF16, tag="e")
                    nc.scalar.activation(
                        out=e, in_=sps,
                        func=mybir.ActivationFunctionType.Exp,
                        scale=float(SCALE),
                    )
                    for h in range(IMW // MMF):
                        nc.tensor.matmul(
                            yps[:, h * MMF:(h + 1) * MMF],
                            vt[:, jt, :],
                            e[:, h * MMF:(h + 1) * MMF],
                            start=(jt == 0),
                            stop=(jt == NJ - 1),
                        )
                    # pairwise bf16 tree for the softmax denominators
                    cur, lvl = e, 0
                    with nc.allow_low_precision("softmax denom tree in bf16"):
                        while levels[lvl] is not None:
                            nxt = treep.tile([P, IMW], BF16, tag=f"tree{lvl}")
                            nc.vector.tensor_add(nxt, levels[lvl], cur)
                            levels[lvl] = None
                            cur = nxt
                            lvl += 1
                    levels[lvl] = cur
                total = levels[5]
                assert total is not None
                # sum over in-tile j (partition axis), broadcast to all rows
                s_bc = normp.tile([P, IMW], F32, tag="sbc")
                nc.gpsimd.partition_all_reduce(
                    s_bc, total, channels=P, reduce_op=bass_isa.ReduceOp.add
                )
                r_bc = normp.tile([P, IMW], F32, tag="rbc")
                r_scr = normp.tile([P, IMW], F32, tag="rscr")
                nc.vector.reciprocal_approx_accurate(r_bc, s_bc, scratch=r_scr)
                y_sb = big.tile([P, IMW], F32R, tag=f"ysb{im}")
                with nc.allow_low_precision("y normalized into f32r"):
                    nc.vector.tensor_mul(y_sb, yps, r_bc)
                y_tiles.append(y_sb)
            emit_proj(NMACRO - 1)
    nc.compile()
    return nc


_CACHED_NC = None


def _get_nc():
    global _CACHED_NC
    if _CACHED_NC is None:
        _CACHED_NC = build_bass()
    return _CACHED_NC


def _prep_in_maps(x, w_qkv, b_qkv, w_out, b_out):
    xs = np.ascontiguousarray(np.asarray(x, np.float32).reshape(8, CH, N))
    wqkvT = np.ascontiguousarray(np.asarray(w_qkv, np.float32).T)
    bv = np.ascontiguousarray(
        np.asarray(b_qkv, np.float32)[2 * IC:3 * IC].reshape(IC, 1)
    )
    woutT = np.ascontiguousarray(np.asarray(w_out, np.float32).T)
    bout = np.ascontiguousarray(np.asarray(b_out, np.float32).reshape(CH, 1))
    return [
        {
            "x": np.ascontiguousarray(xs[i]),
            "wqkvT": wqkvT,
            "bv": bv,
            "woutT": woutT,
            "bout": bout,
        }
        for i in range(8)
    ]


def kernel(x, w_qkv, b_qkv, w_out, b_out, _trace=False, _trace_kwargs=None):
    nc = _get_nc()
    in_maps = _prep_in_maps(x, w_qkv, b_qkv, w_out, b_out)
    res = run_bass_kernel_spmd(
        nc, in_maps, core_ids=list(range(8)), trace=_trace,
        **(_trace_kwargs or {}),
    )
    out = np.stack([res.results[i]["out"] for i in range(8)])
    out = out.reshape(8, CH, 64, 64).astype(np.float32)
    if _trace:
        return out, res
    return out


if __name__ == "__main__":
    rng = np.random.default_rng(0)
    x = rng.standard_normal((8, CH, 64, 64), dtype=np.float32)
    w_qkv = (rng.standard_normal((3 * IC, CH), dtype=np.float32) * 0.01)
    b_qkv = (rng.standard_normal((3 * IC,), dtype=np.float32) * 0.01)
    w_out = (rng.standard_normal((CH, IC), dtype=np.float32) * 0.01)
    b_out = (rng.standard_normal((CH,), dtype=np.float32) * 0.01)
    o = kernel(x, w_qkv=w_qkv, b_qkv=b_qkv, w_out=w_out, b_out=b_out)
    print(o.shape, o.dtype)


# revision 41
# speedup vs baseline: 1.0081x; 1.0081x over previous
"""Trainium2 Bass kernel for DisentangledSpatialSA.

Reference computation (per batch b, with C=256, IC=128, N=64*64=4096):
    qkv = w_qkv @ x + b_qkv                    # [384, N]
    q, k, v = qkv split into 3 x [IC, N]
    k -= mean_n(k); q -= mean_n(q)             # per-channel spatial centering
    pw[i, j] = sum_c k[c, i] * q[c, j]
    pw = softmax(pw / (sqrt(IC) * TEMP), axis=j)
    y[c, i] = sum_j pw[i, j] * v[c, j]
    out = x + w_out @ y + b_out

Simplifications used (exact up to softmax shift invariance):
  - q centering and the q/k biases cancel inside the row softmax, so only k
    is centered and only v's bias is applied.
  - softmax max-subtraction is skipped: logits are ~N(0, 0.5), safely inside
    fp32 exp range.
  - normalization is applied after the PV matmul: y = (V e) / s, with the
    row sums s computed by a bf16 pairwise tree on VectorE plus one
    gpsimd.partition_all_reduce (which also broadcasts across partitions).

Sharding: data-parallel over batch, one batch element per NeuronCore (8).

Layout: everything channel-major with spatial flattened (n = 4096).
S_t[j, i] tiles are built with keys j on partitions (lhsT = q-tile, rhs = k~),
so the softmax denominators are partition-axis sums; PV uses lhsT = v^T tiles
(DMA transpose of bf16 v) and rhs = exp(S_t).
"""

import numpy as np

import concourse.bacc as bacc
import concourse.bass as bass
import concourse.tile as tile
from concourse import mybir
from concourse import bass_isa
from concourse.bass_utils import run_bass_kernel_spmd
from concourse.masks import make_identity

F32 = mybir.dt.float32
F32R = mybir.dt.float32r
BF16 = mybir.dt.bfloat16

CH = 256
IC = 128
N = 4096
TEMP = 0.05
SCALE = 1.0 / (np.sqrt(np.float32(IC)) * TEMP)  # applied inside exp

P = 128          # partitions
IMW = 1024       # i-macro tile width (query free dim per attention pass)
NMACRO = N // IMW
NJ = N // P      # 32 key tiles
MMF = 512        # max moving free dim per matmul


def build_bass() -> bass.Bass:
    nc = bacc.Bacc("TRN2", target_bir_lowering=False, debug=False, num_devices=8)

    # fp32r-typed external inputs: bits are fp32; fp32r lets matmuls consume
    # them at full (1 cycle/row) rate without an on-chip rounding pass.
    x_d = nc.dram_tensor("x", [CH, N], F32R, kind="ExternalInput")
    wqkvT_d = nc.dram_tensor("wqkvT", [CH, 3 * IC], F32R, kind="ExternalInput")
    bv_d = nc.dram_tensor("bv", [IC, 1], F32, kind="ExternalInput")
    woutT_d = nc.dram_tensor("woutT", [IC, CH], F32R, kind="ExternalInput")
    bout_row_d = nc.dram_tensor("bout_row", [2, IC], F32R, kind="ExternalInput")
    out_d = nc.dram_tensor("out", [CH, N], F32, kind="ExternalOutput")

    with tile.TileContext(nc) as tc:
        with (
            tc.tile_pool(name="big", bufs=1) as big,          # long-lived SBUF
            tc.tile_pool(name="small", bufs=1) as small,      # weights/bias
            tc.tile_pool(name="ework", bufs=8) as ework,      # exp tiles
            tc.tile_pool(name="tree", bufs=3) as treep,       # softmax-sum tree
            tc.tile_pool(name="norm", bufs=2) as normp,       # sums/recip
            tc.tile_pool(name="outp", bufs=4) as outp,        # output staging
            tc.tile_pool(name="spsum", bufs=2, space="PSUM") as spsum,  # 4 banks
            tc.tile_pool(name="ypsum", bufs=2, space="PSUM") as ypsum,  # 4 banks
        ):
            # ---------- load inputs ----------
            # Small tensors issue from the Scalar HWDGE queue, bulk x from the
            # Sync queue: dma_start issue costs ~1.3us each, so spreading
            # queues + issuing weights first gets the first matmul going early.
            W = []
            for cchunk in range(2):
                wt = small.tile([P, 3 * IC], F32R, tag=f"w{cchunk}")
                nc.scalar.dma_start(out=wt, in_=wqkvT_d[cchunk * P:(cchunk + 1) * P, :])
                W.append(wt)
            woutT = small.tile([IC, CH], F32R, tag="woutT")
            nc.scalar.dma_start(out=woutT, in_=woutT_d[:, :])
            bv = small.tile([IC, 1], F32, tag="bv")
            nc.scalar.dma_start(out=bv, in_=bv_d[:, :])
            bout_row = []
            for oc in range(2):
                brt = small.tile([1, IC], F32R, tag=f"bout_row{oc}")
                nc.scalar.dma_start(out=brt, in_=bout_row_d[oc:oc + 1, :])
                bout_row.append(brt)
            ones_f32 = small.tile([1, IMW], F32, tag="ones_f32")
            nc.vector.memset(ones_f32, 1.0)
            ones_row = small.tile([1, IMW], F32R, tag="ones_row")
            nc.vector.tensor_copy(ones_row, ones_f32)
            ident_bf = small.tile([P, P], BF16, tag="ident")
            make_identity(nc, ident_bf)
            # ~3.4us of dependency-free matmuls: lifts the PE HAM clock gate
            # to 2.4 GHz before the real work lands
            warm_ps = spsum.tile([P, P], F32, tag="s")
            for _ in range(64):
                nc.tensor.matmul(warm_ps, ident_bf, ident_bf, start=True, stop=True)
            X = []
            for cchunk in range(2):
                xt = big.tile([P, N], F32R, tag=f"x{cchunk}")
                for h in range(2):
                    sl = slice(h * (N // 2), (h + 1) * (N // 2))
                    nc.sync.dma_start(
                        out=xt[:, sl], in_=x_d[cchunk * P:(cchunk + 1) * P, sl]
                    )
                X.append(xt)

            # ---------- QKV projection; order chosen so the attention loop's
            # dependencies (k -> centered k, v -> v^T, then q) finish earliest
            q_sb = big.tile([P, N], BF16, tag="q")
            k_sb = big.tile([P, N], F32, tag="k")
            v_bf = big.tile([P, N], BF16, tag="v")
            vt = big.tile([P, NJ, IC], BF16, tag="vt")
            kc_sb = big.tile([P, N], BF16, tag="kc")
            ksum8 = small.tile([P, N // MMF], F32, tag="ksum8")

            def qkv_pass(m):
                for nt in range(N // MMF):
                    pool = ypsum if nt % 2 == 0 else spsum
                    ps = pool.tile([P, MMF], F32, tag="ypsum" if nt % 2 == 0 else "s")
                    sl = slice(nt * MMF, (nt + 1) * MMF)
                    for cchunk in range(2):
                        nc.tensor.matmul(
                            ps,
                            W[cchunk][:, m * IC:(m + 1) * IC],
                            X[cchunk][:, sl],
                            start=(cchunk == 0),
                            stop=(cchunk == 1),
                        )
                    if m == 0:
                        with nc.allow_low_precision("q used in bf16 logits"):
                            nc.vector.tensor_copy(q_sb[:, sl], ps)
                    elif m == 1:
                        nc.vector.tensor_copy(k_sb[:, sl], ps)
                        # fold the spatial sum into the pipeline per chunk
                        nc.vector.tensor_reduce(
                            out=ksum8[:, nt:nt + 1], in_=ps,
                            axis=mybir.AxisListType.X, op=mybir.AluOpType.add,
                        )
                    else:
                        nc.scalar.activation(
                            out=v_bf[:, sl], in_=ps,
                            func=mybir.ActivationFunctionType.Identity,
                            bias=bv, scale=1.0,
                        )
                        # v^T tiles via PE transpose as each chunk lands
                        for jt in range(nt * MMF // P, (nt + 1) * MMF // P):
                            tps = spsum.tile([P, P], BF16, tag="s")
                            nc.tensor.transpose(
                                tps, v_bf[:, jt * P:(jt + 1) * P], ident_bf
                            )
                            nc.vector.tensor_copy(vt[:, jt, :], tps)

            qkv_pass(1)  # k
            # ---------- center k over spatial axis (write bf16) ----------
            kneg = small.tile([P, 1], F32, tag="kneg")
            nc.vector.tensor_reduce(
                out=kneg, in_=ksum8, axis=mybir.AxisListType.X, op=mybir.AluOpType.add
            )
            nc.vector.tensor_scalar_mul(kneg, kneg, -1.0 / N)
            for h in range(4):
                sl = slice(h * (N // 4), (h + 1) * (N // 4))
                nc.scalar.activation(
                    out=kc_sb[:, sl], in_=k_sb[:, sl],
                    func=mybir.ActivationFunctionType.Identity,
                    bias=kneg, scale=1.0,
                )
            qkv_pass(2)  # v (+ transposes)
            qkv_pass(0)  # q

            # ---------- output projection (emitted per-imacro, interleaved
            # into the NEXT imacro's loop so it never head-of-line blocks PE)
            y_tiles = []

            def emit_proj(im):
                # out = x + woutT.T @ y + bias; bias folded in as a K=1 matmul
                # so the whole epilogue needs no ScalarE work
                isl = slice(im * IMW, (im + 1) * IMW)
                for oc in range(2):
                    pps = ypsum.tile([P, IMW], F32, tag="ypsum")
                    for h in range(IMW // MMF):
                        nc.tensor.matmul(
                            pps[:, h * MMF:(h + 1) * MMF],
                            woutT[:, oc * P:(oc + 1) * P],
                            y_tiles[im][:, h * MMF:(h + 1) * MMF],
                            start=True,
                            stop=False,
                        )
                        nc.tensor.matmul(
                            pps[:, h * MMF:(h + 1) * MMF],
                            bout_row[oc],
                            ones_row[:, h * MMF:(h + 1) * MMF],
                            start=False,
                            stop=True,
                        )
                    osb = outp.tile([P, IMW], F32, tag="osb")
                    nc.vector.tensor_add(osb, pps, X[oc][:, isl].bitcast(F32))
                    nc.sync.dma_start(out=out_d[oc * P:(oc + 1) * P, isl], in_=osb)

            # ---------- attention (normalized y saved; projection deferred) --
            for im in range(NMACRO):
                yps = ypsum.tile([P, IMW], F32, tag="ypsum")
                levels: list = [None] * 8
                for jt in range(NJ):
                    sps = spsum.tile([P, IMW], F32, tag="s")
                    for h in range(IMW // MMF):
                        nc.tensor.matmul(
                            sps[:, h * MMF:(h + 1) * MMF],
                            q_sb[:, jt * P:(jt + 1) * P],
                            kc_sb[:, im * IMW + h * MMF: im * IMW + (h + 1) * MMF],
                            start=True,
                            stop=True,
                        )
                    e = ework.tile([P, IMW], BF16, tag="e")
                    nc.scalar.activation(
                        out=e, in_=sps,
                        func=mybir.ActivationFunctionType.Exp,
                        scale=float(SCALE),
                    )
                    for h in range(IMW // MMF):
                        nc.tensor.matmul(
                            yps[:, h * MMF:(h + 1) * MMF],
                            vt[:, jt, :],
                            e[:, h * MMF:(h + 1) * MMF],
                            start=(jt == 0),
                            stop=(jt == NJ - 1),
                        )
                    # pairwise bf16 tree for the softmax denominators
                    cur, lvl = e, 0
                    with nc.allow_low_precision("softmax denom tree in bf16"):
                        while levels[lvl] is not None:
                            nxt = treep.tile([P, IMW], BF16, tag=f"tree{lvl}")
                            nc.vector.tensor_add(nxt, levels[lvl], cur)
                            levels[lvl] = None
                            cur = nxt
                            lvl += 1
                    levels[lvl] = cur
                total = levels[5]
                assert total is not None
                # sum over in-tile j (partition axis), broadcast to all rows
                s_bc = normp.tile([P, IMW], F32, tag="sbc")
                nc.gpsimd.partition_all_reduce(
                    s_bc, total, channels=P, reduce_op=bass_isa.ReduceOp.add
                )
                r_bc = normp.tile([P, IMW], F32, tag="rbc")
                r_scr = normp.tile([P, IMW], F32, tag="rscr")
                nc.vector.reciprocal_approx_accurate(r_bc, s_bc, scratch=r_scr)
                y_sb = big.tile([P, IMW], F32R, tag=f"ysb{im}")
                with nc.allow_low_precision("y normalized into f32r"):
                    nc.vector.tensor_mul(y_sb, yps, r_bc)
                y_tiles.append(y_sb)
            for im in range(NMACRO):
                emit_proj(im)
    nc.compile()
    return nc


_CACHED_NC = None


def _get_nc():
    global _CACHED_NC
    if _CACHED_NC is None:
        _CACHED_NC = build_bass()
    return _CACHED_NC


def _prep_in_maps(x, w_qkv, b_qkv, w_out, b_out):
    xs = np.ascontiguousarray(np.asarray(x, np.float32).reshape(8, CH, N))
    wqkvT = np.ascontiguousarray(np.asarray(w_qkv, np.float32).T)
    bv = np.ascontiguousarray(
        np.asarray(b_qkv, np.float32)[2 * IC:3 * IC].reshape(IC, 1)
    )
    woutT = np.ascontiguousarray(np.asarray(w_out, np.float32).T)
    bout_row = np.ascontiguousarray(np.asarray(b_out, np.float32).reshape(2, IC))
    return [
        {
            "x": np.ascontiguousarray(xs[i]),
            "wqkvT": wqkvT,
            "bv": bv,
            "woutT": woutT,
            "bout_row": bout_row,
        }
        for i in range(8)
    ]


def kernel(x, w_qkv, b_qkv, w_out, b_out, _trace=False, _trace_kwargs=None):
    nc = _get_nc()
    in_maps = _prep_in_maps(x, w_qkv, b_qkv, w_out, b_out)
    res = run_bass_kernel_spmd(
        nc, in_maps, core_ids=list(range(8)), trace=_trace,
        **(_trace_kwargs or {}),
    )
    out = np.stack([res.results[i]["out"] for i in range(8)])
    out = out.reshape(8, CH, 64, 64).astype(np.float32)
    if _trace:
        return out, res
    return out


if __name__ == "__main__":
    rng = np.random.default_rng(0)
    x = rng.standard_normal((8, CH, 64, 64), dtype=np.float32)
    w_qkv = (rng.standard_normal((3 * IC, CH), dtype=np.float32) * 0.01)
    b_qkv = (rng.standard_normal((3 * IC,), dtype=np.float32) * 0.01)
    w_out = (rng.standard_normal((CH, IC), dtype=np.float32) * 0.01)
    b_out = (rng.standard_normal((CH,), dtype=np.float32) * 0.01)
    o = kernel(x, w_qkv=w_qkv, b_qkv=b_qkv, w_out=w_out, b_out=b_out)
    print(o.shape, o.dtype)


# revision 46
# speedup vs baseline: 1.0543x; 1.0458x over previous
"""Trainium2 Bass kernel for DisentangledSpatialSA.

Reference computation (per batch b, with C=256, IC=128, N=64*64=4096):
    qkv = w_qkv @ x + b_qkv                    # [384, N]
    q, k, v = qkv split into 3 x [IC, N]
    k -= mean_n(k); q -= mean_n(q)             # per-channel spatial centering
    pw[i, j] = sum_c k[c, i] * q[c, j]
    pw = softmax(pw / (sqrt(IC) * TEMP), axis=j)
    y[c, i] = sum_j pw[i, j] * v[c, j]
    out = x + w_out @ y + b_out

Simplifications used (exact up to softmax shift invariance):
  - q centering and the q/k biases cancel inside the row softmax, so only k
    is centered and only v's bias is applied.
  - softmax max-subtraction is skipped: logits are ~N(0, 0.5), safely inside
    fp32 exp range.
  - normalization is applied after the PV matmul: y = (V e) / s, with the
    row sums s computed by a bf16 pairwise tree on VectorE plus one
    gpsimd.partition_all_reduce (which also broadcasts across partitions).

Sharding: data-parallel over batch, one batch element per NeuronCore (8).

Layout: everything channel-major with spatial flattened (n = 4096).
S_t[j, i] tiles are built with keys j on partitions (lhsT = q-tile, rhs = k~),
so the softmax denominators are partition-axis sums; PV uses lhsT = v^T tiles
(DMA transpose of bf16 v) and rhs = exp(S_t).
"""

import numpy as np

import concourse.bacc as bacc
import concourse.bass as bass
import concourse.tile as tile
from concourse import mybir
from concourse import bass_isa
from concourse.bass_utils import run_bass_kernel_spmd
from concourse.masks import make_identity

F32 = mybir.dt.float32
F32R = mybir.dt.float32r
BF16 = mybir.dt.bfloat16

CH = 256
IC = 128
N = 4096
TEMP = 0.05
SCALE = 1.0 / (np.sqrt(np.float32(IC)) * TEMP)  # applied inside exp

P = 128          # partitions
IMW = 1024       # i-macro tile width (query free dim per attention pass)
NMACRO = N // IMW
NJ = N // P      # 32 key tiles
MMF = 512        # max moving free dim per matmul


def build_bass() -> bass.Bass:
    nc = bacc.Bacc("TRN2", target_bir_lowering=False, debug=False, num_devices=8)

    # fp32r-typed external inputs: bits are fp32; fp32r lets matmuls consume
    # them at full (1 cycle/row) rate without an on-chip rounding pass.
    x_d = nc.dram_tensor("x", [CH, N], F32R, kind="ExternalInput")
    wqkvT_d = nc.dram_tensor("wqkvT", [CH, 3 * IC], F32R, kind="ExternalInput")
    bv_d = nc.dram_tensor("bv", [IC, 1], F32, kind="ExternalInput")
    woutT_d = nc.dram_tensor("woutT", [IC, CH], F32R, kind="ExternalInput")
    bout_row_d = nc.dram_tensor("bout_row", [2, IC], F32R, kind="ExternalInput")
    out_d = nc.dram_tensor("out", [CH, N], F32, kind="ExternalOutput")

    with tile.TileContext(nc) as tc:
        with (
            tc.tile_pool(name="big", bufs=1) as big,          # long-lived SBUF
            tc.tile_pool(name="small", bufs=1) as small,      # weights/bias
            tc.tile_pool(name="ework", bufs=8) as ework,      # exp tiles
            tc.tile_pool(name="tree", bufs=3) as treep,       # softmax-sum tree
            tc.tile_pool(name="norm", bufs=2) as normp,       # sums/recip
            tc.tile_pool(name="outp", bufs=4) as outp,        # output staging
            tc.tile_pool(name="spsum", bufs=2, space="PSUM") as spsum,  # 4 banks
            tc.tile_pool(name="ypsum", bufs=4, space="PSUM") as ypsum,  # 4 banks
        ):
            # ---------- load inputs ----------
            # Small tensors issue from the Scalar HWDGE queue, bulk x from the
            # Sync queue: dma_start issue costs ~1.3us each, so spreading
            # queues + issuing weights first gets the first matmul going early.
            W = []
            for cchunk in range(2):
                wt = small.tile([P, 3 * IC], F32R, tag=f"w{cchunk}")
                nc.scalar.dma_start(out=wt, in_=wqkvT_d[cchunk * P:(cchunk + 1) * P, :])
                W.append(wt)
            woutT = small.tile([IC, CH], F32R, tag="woutT")
            nc.scalar.dma_start(out=woutT, in_=woutT_d[:, :])
            bv = small.tile([IC, 1], F32, tag="bv")
            nc.scalar.dma_start(out=bv, in_=bv_d[:, :])
            bout_row = []
            for oc in range(2):
                brt = small.tile([1, IC], F32R, tag=f"bout_row{oc}")
                nc.scalar.dma_start(out=brt, in_=bout_row_d[oc:oc + 1, :])
                bout_row.append(brt)
            ones_f32 = small.tile([1, IMW], F32, tag="ones_f32")
            nc.vector.memset(ones_f32, 1.0)
            ones_row = small.tile([1, IMW], F32R, tag="ones_row")
            nc.vector.tensor_copy(ones_row, ones_f32)
            ident_bf = small.tile([P, P], BF16, tag="ident")
            make_identity(nc, ident_bf)
            # ~3.4us of dependency-free matmuls: lifts the PE HAM clock gate
            # to 2.4 GHz before the real work lands
            warm_ps = spsum.tile([P, P], F32, tag="s")
            for _ in range(40):
                nc.tensor.matmul(warm_ps, ident_bf, ident_bf, start=True, stop=True)
            X = []
            for cchunk in range(2):
                xt = big.tile([P, N], F32R, tag=f"x{cchunk}")
                for h in range(2):
                    sl = slice(h * (N // 2), (h + 1) * (N // 2))
                    nc.sync.dma_start(
                        out=xt[:, sl], in_=x_d[cchunk * P:(cchunk + 1) * P, sl]
                    )
                X.append(xt)

            # ---------- QKV projection; order chosen so the attention loop's
            # dependencies (k -> centered k, v -> v^T, then q) finish earliest
            q_sb = big.tile([P, N], BF16, tag="q")
            k_sb = big.tile([P, N], F32, tag="k")
            v_bf = big.tile([P, N], BF16, tag="v")
            vt = big.tile([P, NJ, IC], BF16, tag="vt")
            kc_sb = big.tile([P, N], BF16, tag="kc")
            ksum8 = small.tile([P, N // MMF], F32, tag="ksum8")

            def qkv_chunk(m, nt, alt_pool=True):
                use_s = alt_pool and nt % 2 == 1
                pool = spsum if use_s else ypsum
                ps = pool.tile([P, MMF], F32, tag="s" if use_s else "ypsum")
                sl = slice(nt * MMF, (nt + 1) * MMF)
                for cchunk in range(2):
                    nc.tensor.matmul(
                        ps,
                        W[cchunk][:, m * IC:(m + 1) * IC],
                        X[cchunk][:, sl],
                        start=(cchunk == 0),
                        stop=(cchunk == 1),
                    )
                if m == 0:
                    with nc.allow_low_precision("q used in bf16 logits"):
                        nc.vector.tensor_copy(q_sb[:, sl], ps)
                elif m == 1:
                    nc.vector.tensor_copy(k_sb[:, sl], ps)
                    # fold the spatial sum into the pipeline per chunk
                    nc.vector.tensor_reduce(
                        out=ksum8[:, nt:nt + 1], in_=ps,
                        axis=mybir.AxisListType.X, op=mybir.AluOpType.add,
                    )
                else:
                    nc.scalar.activation(
                        out=v_bf[:, sl], in_=ps,
                        func=mybir.ActivationFunctionType.Identity,
                        bias=bv, scale=1.0,
                    )
                    # v^T tiles via PE transpose as each chunk lands
                    for jt in range(nt * MMF // P, (nt + 1) * MMF // P):
                        tps = ypsum.tile([P, P], BF16, tag="ypsum")
                        nc.tensor.transpose(
                            tps, v_bf[:, jt * P:(jt + 1) * P], ident_bf
                        )
                        nc.vector.tensor_copy(vt[:, jt, :], tps)

            # k projection + centering fully before attention (the softmax
            # logits need the global spatial mean of k)
            for nt in range(N // MMF):
                qkv_chunk(1, nt)
            kneg = small.tile([P, 1], F32, tag="kneg")
            nc.vector.tensor_reduce(
                out=kneg, in_=ksum8, axis=mybir.AxisListType.X, op=mybir.AluOpType.add
            )
            nc.vector.tensor_scalar_mul(kneg, kneg, -1.0 / N)
            for h in range(4):
                sl = slice(h * (N // 4), (h + 1) * (N // 4))
                nc.scalar.activation(
                    out=kc_sb[:, sl], in_=k_sb[:, sl],
                    func=mybir.ActivationFunctionType.Identity,
                    bias=kneg, scale=1.0,
                )
            # first q / v chunks (+ their v^T tiles): just enough for the
            # attention loop to launch; the rest streams into imacro 0's loop
            qkv_chunk(0, 0, alt_pool=False)
            qkv_chunk(2, 0, alt_pool=False)

            # ---------- output projection (emitted per-imacro, interleaved
            # into the NEXT imacro's loop so it never head-of-line blocks PE)
            y_tiles = []

            def emit_proj(im):
                # out = x + woutT.T @ y + bias; bias folded in as a K=1 matmul
                # so the whole epilogue needs no ScalarE work
                isl = slice(im * IMW, (im + 1) * IMW)
                for oc in range(2):
                    osb = outp.tile([P, IMW], F32, tag="osb")
                    for h in range(IMW // MMF):
                        pps = ypsum.tile([P, MMF], F32, tag="ypsum")
                        nc.tensor.matmul(
                            pps,
                            woutT[:, oc * P:(oc + 1) * P],
                            y_tiles[im][:, h * MMF:(h + 1) * MMF],
                            start=True,
                            stop=False,
                        )
                        nc.tensor.matmul(
                            pps,
                            bout_row[oc],
                            ones_row[:, h * MMF:(h + 1) * MMF],
                            start=False,
                            stop=True,
                        )
                        nc.vector.tensor_add(
                            osb[:, h * MMF:(h + 1) * MMF], pps,
                            X[oc][:, im * IMW + h * MMF: im * IMW + (h + 1) * MMF].bitcast(F32),
                        )
                    nc.sync.dma_start(out=out_d[oc * P:(oc + 1) * P, isl], in_=osb)

            # ---------- attention; imacro 0 also streams in the remaining
            # q/v projection chunks and v^T transposes ----------
            for im in range(NMACRO):
                yhalf = [
                    ypsum.tile([P, MMF], F32, tag="ypsum", name=f"yh{im}_{h}")
                    for h in range(IMW // MMF)
                ]
                levels: list = [None] * 8
                for jt in range(NJ):
                    if im == 0:
                        if jt <= (N // MMF) - 2:
                            qkv_chunk(2, jt + 1, alt_pool=False)  # v chunk
                        if jt % 4 == 1 and (jt + 3) // 4 <= (N // MMF) - 1:
                            qkv_chunk(0, (jt + 3) // 4, alt_pool=False)  # q
                    sps = spsum.tile([P, IMW], F32, tag="s")
                    for h in range(IMW // MMF):
                        nc.tensor.matmul(
                            sps[:, h * MMF:(h + 1) * MMF],
                            q_sb[:, jt * P:(jt + 1) * P],
                            kc_sb[:, im * IMW + h * MMF: im * IMW + (h + 1) * MMF],
                            start=True,
                            stop=True,
                        )
                    e = ework.tile([P, IMW], BF16, tag="e")
                    nc.scalar.activation(
                        out=e, in_=sps,
                        func=mybir.ActivationFunctionType.Exp,
                        scale=float(SCALE),
                    )
                    for h in range(IMW // MMF):
                        nc.tensor.matmul(
                            yhalf[h],
                            vt[:, jt, :],
                            e[:, h * MMF:(h + 1) * MMF],
                            start=(jt == 0),
                            stop=(jt == NJ - 1),
                        )
                    # pairwise bf16 tree for the softmax denominators
                    cur, lvl = e, 0
                    with nc.allow_low_precision("softmax denom tree in bf16"):
                        while levels[lvl] is not None:
                            nxt = treep.tile([P, IMW], BF16, tag=f"tree{lvl}")
                            nc.vector.tensor_add(nxt, levels[lvl], cur)
                            levels[lvl] = None
                            cur = nxt
                            lvl += 1
                    levels[lvl] = cur
                total = levels[5]
                assert total is not None
                # sum over in-tile j (partition axis), broadcast to all rows
                s_bc = normp.tile([P, IMW], F32, tag="sbc")
                nc.gpsimd.partition_all_reduce(
                    s_bc, total, channels=P, reduce_op=bass_isa.ReduceOp.add
                )
                r_bc = normp.tile([P, IMW], F32, tag="rbc")
                r_scr = normp.tile([P, IMW], F32, tag="rscr")
                nc.vector.reciprocal_approx_accurate(r_bc, s_bc, scratch=r_scr)
                y_sb = big.tile([P, IMW], F32R, tag=f"ysb{im}")
                with nc.allow_low_precision("y normalized into f32r"):
                    for h in range(IMW // MMF):
                        nc.vector.tensor_mul(
                            y_sb[:, h * MMF:(h + 1) * MMF], yhalf[h],
                            r_bc[:, h * MMF:(h + 1) * MMF],
                        )
                y_tiles.append(y_sb)
            for im in range(NMACRO):
                emit_proj(im)
    nc.compile()
    return nc


_CACHED_NC = None


def _get_nc():
    global _CACHED_NC
    if _CACHED_NC is None:
        _CACHED_NC = build_bass()
    return _CACHED_NC


def _prep_in_maps(x, w_qkv, b_qkv, w_out, b_out):
    xs = np.ascontiguousarray(np.asarray(x, np.float32).reshape(8, CH, N))
    wqkvT = np.ascontiguousarray(np.asarray(w_qkv, np.float32).T)
    bv = np.ascontiguousarray(
        np.asarray(b_qkv, np.float32)[2 * IC:3 * IC].reshape(IC, 1)
    )
    woutT = np.ascontiguousarray(np.asarray(w_out, np.float32).T)
    bout_row = np.ascontiguousarray(np.asarray(b_out, np.float32).reshape(2, IC))
    return [
        {
            "x": np.ascontiguousarray(xs[i]),
            "wqkvT": wqkvT,
            "bv": bv,
            "woutT": woutT,
            "bout_row": bout_row,
        }
        for i in range(8)
    ]


def kernel(x, w_qkv, b_qkv, w_out, b_out, _trace=False, _trace_kwargs=None):
    nc = _get_nc()
    in_maps = _prep_in_maps(x, w_qkv, b_qkv, w_out, b_out)
    res = run_bass_kernel_spmd(
        nc, in_maps, core_ids=list(range(8)), trace=_trace,
        **(_trace_kwargs or {}),
    )
    out = np.stack([res.results[i]["out"] for i in range(8)])
    out = out.reshape(8, CH, 64, 64).astype(np.float32)
    if _trace:
        return out, res
    return out


if __name__ == "__main__":
    rng = np.random.default_rng(0)
    x = rng.standard_normal((8, CH, 64, 64), dtype=np.float32)
    w_qkv = (rng.standard_normal((3 * IC, CH), dtype=np.float32) * 0.01)
    b_qkv = (rng.standard_normal((3 * IC,), dtype=np.float32) * 0.01)
    w_out = (rng.standard_normal((CH, IC), dtype=np.float32) * 0.01)
    b_out = (rng.standard_normal((CH,), dtype=np.float32) * 0.01)
    o = kernel(x, w_qkv=w_qkv, b_qkv=b_qkv, w_out=w_out, b_out=b_out)
    print(o.shape, o.dtype)
